# revision 25
# baseline (speedup 1.0000x reference)
"""Trainium2 Bass kernel for nn_MultiHeadAttention_89524298317897 (v10).

Data-parallel over batch: core b computes batch element b end-to-end.
All on-device tensors bf16 (host pre-casts + pre-transposes); PSUM
accumulation fp32.

Math per core (batch b), faithful to torch's .view head split (chunks
the sequence dim): head h token t <-> qp[64h + t//16, (t%16)*64 + d].
Token order inside the kernel is the fixed permutation
t = row*16 + 2c + j  ->  global key id g = j*512 + c*64 + row;
attention is permutation invariant and the output eviction un-permutes.

Layouts (partition p first):
  qS2 [128, 16, 512]    qS2[64*pi + d, h, c*64+row] = qp^T for query half
                        pi = (t%16)%2 (only the diagonal halves needed)
  kS3 [128, 16, 1024]   per partition half jj: f'' in [0,512) = native
                        half-jj keys (local order c*64+row), [512,1024) =
                        the other half's keys (same local order, filled
                        by 2 swap DMAs). Key id from half pi's view:
                        g = (f'' + 512*pi) % 1024.
  vS  [128, 16, 8, 64]  vS[g%128, h, g//128, d] = vp value for key g
  ET  [128, 8, 2, 512]  exp(scores): ET[p, kc, pi, q'] keys g =
                        (kc*128+p+512*pi)%1024, queries (pi, q'=c*64+row)
  attn[128, 8, 1024]    out^T: attn[o%128, o//128, s]

Per head: scores = 8 key-chunks x 2 row-tiles (K=64 halves, M=128 keys,
N=512 queries) -> exp (ACT) -> PV: 8 accumulating col-2x pairs (K=128
keys, M=64 d) using vS chunk (kc+4*pi)%8 -> Z: DVE partial-sums of ET
over chunks (bf16) then ONE ones-matmul pair -> DVE reciprocal +
multiply eviction.  v-projection and output-projection tiles interleave
between heads so the PE stays busy while ACT computes exp.
"""
import os
import sys

for _p in ("/opt/trn_rl_repo",):
    if os.path.isdir(_p) and _p not in sys.path:
        sys.path.insert(0, _p)

import numpy as np
import ml_dtypes
import concourse.bass as bass
import concourse.mybir as mybir
import concourse.tile as tile
from concourse import bacc
from concourse.bass_utils import run_bass_kernel_spmd

B, S, D, NH, DH = 8, 1024, 1024, 16, 64
P = 128
F32 = mybir.dt.float32
BF16 = mybir.dt.bfloat16
EXP_FN = mybir.ActivationFunctionType.Exp
ADD = mybir.AluOpType.add
MULT = mybir.AluOpType.mult

_CACHE: dict = {}


def _build_nc():
    nc = bacc.Bacc("TRN2", target_bir_lowering=False, debug=False)

    qT = nc.dram_tensor("qT", [D, S], BF16, kind="ExternalInput")
    kT = nc.dram_tensor("kT", [D, S], BF16, kind="ExternalInput")
    vT = nc.dram_tensor("vT", [D, S], BF16, kind="ExternalInput")
    wqT = nc.dram_tensor("wqT", [D, D], BF16, kind="ExternalInput")
    wkT = nc.dram_tensor("wkT", [D, D], BF16, kind="ExternalInput")
    wvT = nc.dram_tensor("wvT", [D, D], BF16, kind="ExternalInput")
    woT = nc.dram_tensor("woT", [D, D], BF16, kind="ExternalInput")
    out = nc.dram_tensor("out", [S, D], F32, kind="ExternalOutput")

    def part3(dram):  # [1024, X] -> [128, 8, X] with row = io*128 + p
        return dram[:].rearrange("(io p) x -> p io x", p=P)

    with tile.TileContext(nc) as tc:
        with tc.tile_pool(name="big", bufs=1) as big, \
             tc.tile_pool(name="pa_x", bufs=2) as pa_x, \
             tc.tile_pool(name="pa_w", bufs=2) as pa_w, \
             tc.tile_pool(name="pet", bufs=2) as pet, \
             tc.tile_pool(name="pes", bufs=3) as pes, \
             tc.tile_pool(name="psb", bufs=2) as psb, \
             tc.tile_pool(name="pconst", bufs=1) as pconst, \
             tc.tile_pool(name="pss", bufs=3, space="PSUM") as pss, \
             tc.tile_pool(name="pnz", bufs=2, space="PSUM") as pnz:

            qS2 = big.tile([P, NH, 512], BF16)
            kS3 = big.tile([P, NH, S], BF16)
            vS = big.tile([P, NH, 8, DH], BF16)
            attn = big.tile([P, 8, S], BF16)
            out3 = out[:].rearrange("(sc p) o -> p sc o", p=P)

            ones_bf = pconst.tile([P, 64], BF16)
            nc.gpsimd.memset(ones_bf[:], 1.0)
            # warm-up burst: lift the HAM clock gate to 8/8 before the
            # first real matmuls (which are gated on input DMA anyway)
            warm = pconst.tile([P, 64], BF16)
            nc.vector.memset(warm[:], 0.0)
            wps = pss.tile([P, 1024], F32, tag="sc", name="wps")
            for i in range(64):
                nc.tensor.matmul(wps[0:64, 0:64], warm[:], warm[:],
                                 start=True, stop=True, skip_group_check=True)

            # ---------- input DMAs (two queues: sync for q, scalar for k)
            xq = pa_x.tile([P, 8, S], BF16, tag="x")
            wq = pa_w.tile([P, 8, D], BF16, tag="w")
            xk = pa_x.tile([P, 8, S], BF16, tag="x")
            wk = pa_w.tile([P, 8, D], BF16, tag="w")
            # all input loads on one queue in consumption order: the
            # HBM link (~358 GB/s) is the constraint, not queue count
            for io in range(8):  # chunked so first matmuls start early
                nc.sync.dma_start(wq[:, io, :], part3(wqT)[:, io, :])
                nc.sync.dma_start(xq[:, io, :], part3(qT)[:, io, :])
            for io in range(8):
                nc.sync.dma_start(wk[:, io, :], part3(wkT)[:, io, :])
                nc.sync.dma_start(xk[:, io, :], part3(kT)[:, io, :])

            # ---------- phase A: q/k projections (transposed out) ----------
            def evict_qk(dst, ps, j, nchunk):
                # ps[ch*64+p', f*512 + a*64 + b] = xp^T[(4j+2f+ch)*64+p',
                #   nchunk*512 + a*64 + b]; query/key half = ch, head =
                #   nchunk*8 + a, local idx = (2j+f)*64 + b
                for f in range(2):
                    mt = 2 * j + f
                    nc.vector.tensor_copy(
                        dst[:, nchunk * 8:(nchunk + 1) * 8,
                            mt * 64:(mt + 1) * 64],
                        ps[:, f * 512:(f + 1) * 512].rearrange(
                            "p (a b) -> p a b", a=8))

            def emit_proj_block(xt, wt, dst, j, nchunk):
                ps = pss.tile([P, 1024], F32, tag="sc")
                for f in range(2):
                    mt = 2 * j + f
                    for io in range(8):
                        nc.tensor.matmul(
                            ps[:, f * 512:(f + 1) * 512],
                            wt[:, io, mt * P:(mt + 1) * P],
                            xt[:, io, nchunk * 512:(nchunk + 1) * 512],
                            start=(io == 0), stop=(io == 7),
                            skip_group_check=True)
                evict_qk(dst, ps[:], j, nchunk)

            def emit_swap(nchunk):
                # kS3 swap: other half's native keys into f'' [512,1024)
                hs = slice(nchunk * 8, nchunk * 8 + 8)
                nc.gpsimd.dma_start(kS3[64:128, hs, 512:1024],
                                    kS3[0:64, hs, 0:512])
                nc.gpsimd.dma_start(kS3[0:64, hs, 512:1024],
                                    kS3[64:128, hs, 0:512])

            for j in range(4):
                emit_proj_block(xq, wq, qS2, j, 0)
            for j in range(4):
                emit_proj_block(xk, wk, kS3, j, 0)
            emit_swap(0)
            for j in range(4):
                emit_proj_block(xq, wq, qS2, j, 1)
            # k-projection nchunk=1 blocks are deferred into heads 0..3

            # ---------- v / wo loads ----------
            xv = pa_x.tile([P, 8, S], BF16, tag="x")
            wv = pa_w.tile([P, 8, D], BF16, tag="w")
            for io in range(8):
                nc.sync.dma_start(wv[:, io, :], part3(wvT)[:, io, :])
                nc.sync.dma_start(xv[:, io, :], part3(vT)[:, io, :])
            wo = pa_w.tile([P, 8, D], BF16, tag="w")
            nc.sync.dma_start(wo[:], part3(woT))

            # ---------- helpers emitted inside the head loop ----------
            # v-proj psum: ps[t, o] = vp[mtv*128 + t, o], t = 64*(h%2)+row,
            # o = m*64+d, m = 8*a3+4*a2+2*a1+a0.  Target:
            # vS[64*a1 + row, 2*mtv + (t>=64), 4*a0+2*a3+a2, d]
            def evict_vproj_full(ps, mtv):
                # ps is one [P, 1024] psum tile; 4 copies + 2 shift DMAs
                src = ps.rearrange("p (a3 a2 a1 a0 d) -> p a1 a0 a3 a2 d",
                                   a3=2, a2=2, a1=2, a0=2)
                sc2 = psb.tile([P, 8, DH], BF16, tag="scratch")
                for hl in range(2):  # ps partition half = head parity
                    h = 2 * mtv + hl
                    pr = slice(hl * 64, hl * 64 + 64)
                    dst = vS[:, h].rearrange("p (a0 a3 a2) d -> p a0 a3 a2 d",
                                             a0=2, a3=2, a2=2)
                    # a1 == hl lands on the same partitions: direct copy
                    nc.vector.tensor_copy(dst[pr], src[pr, hl])
                    # a1 != hl needs the 64-partition shift: via scratch
                    nc.vector.tensor_copy(
                        sc2[pr].rearrange("p (a0 a3 a2) d -> p a0 a3 a2 d",
                                          a0=2, a3=2, a2=2),
                        src[pr, 1 - hl])
                other = [slice(64, 128), slice(0, 64)]
                for hl in range(2):
                    pr = slice(hl * 64, hl * 64 + 64)
                    nc.gpsimd.dma_start(
                        vS[other[hl], 2 * mtv + hl].rearrange(
                            "p (a0 a3 a2) d -> p (a0 a3 a2 d)",
                            a0=2, a3=2, a2=2),
                        sc2[pr].rearrange("p c d -> p (c d)"))

            def evict_vproj_halves(halves, mtv):
                # same mapping, but from two separate [P, 512] psum tiles
                # (a3 = which tile); only used for the head-0 pipeline fill
                sc2 = psb.tile([P, 8, DH], BF16, tag="scratch")
                for a3 in range(2):
                    src = halves[a3].rearrange(
                        "p (a2 a1 a0 d) -> p a1 a0 a2 d", a2=2, a1=2, a0=2)
                    for hl in range(2):
                        h = 2 * mtv + hl
                        pr = slice(hl * 64, hl * 64 + 64)
                        dst = vS[:, h].rearrange(
                            "p (a0 a3 a2) d -> p a3 a0 a2 d", a0=2, a3=2,
                            a2=2)
                        nc.vector.tensor_copy(dst[pr, a3], src[pr, hl])
                        scv = sc2[:].rearrange(
                            "p (a0 a3 a2) d -> p a3 a0 a2 d", a0=2, a3=2,
                            a2=2)
                        nc.vector.tensor_copy(scv[pr, a3], src[pr, 1 - hl])
                other = [slice(64, 128), slice(0, 64)]
                for hl in range(2):
                    pr = slice(hl * 64, hl * 64 + 64)
                    nc.gpsimd.dma_start(
                        vS[other[hl], 2 * mtv + hl].rearrange(
                            "p (a0 a3 a2) d -> p (a0 a3 a2 d)",
                            a0=2, a3=2, a2=2),
                        sc2[pr].rearrange("p c d -> p (c d)"))

            def emit_vproj(mtv):
                ps = pss.tile([P, 1024], F32, tag="sc")
                for f in range(2):
                    for io in range(8):
                        nc.tensor.matmul(
                            ps[:, f * 512:(f + 1) * 512],
                            xv[:, io, mtv * P:(mtv + 1) * P],
                            wv[:, io, f * 512:(f + 1) * 512],
                            start=(io == 0), stop=(io == 7))
                evict_vproj_full(ps[:], mtv)

            def emit_phaseC(st, act_evict=False):
                ps = pss.tile([P, 1024], F32, tag="sc")
                for oc in range(2):
                    for jc in range(8):
                        nc.tensor.matmul(
                            ps[:, oc * 512:(oc + 1) * 512],
                            attn[:, jc, st * P:(st + 1) * P],
                            wo[:, jc, oc * 512:(oc + 1) * 512],
                            start=(jc == 0), stop=(jc == 7))
                res = psb.tile([P, 1024], F32, tag="res")
                if act_evict:
                    nc.scalar.activation(
                        res[:], ps[:], mybir.ActivationFunctionType.Copy)
                else:
                    nc.vector.tensor_copy(res[:], ps[:])
                nc.gpsimd.dma_start(out3[:, st, :], res[:])

            # ---------- phase B: software-pipelined head loop ----------
            et_tiles = {}
            es_tiles = {}
            for h in range(NH + 1):
                ET = None
                if h < NH:
                    ET = pet.tile([P, 8, 2, 512], BF16, tag="et")
                    et_tiles[h] = ET
                hp = h - 1
                ETp = et_tiles.pop(hp) if hp >= 0 else None
                v0 = None
                if h == 0:
                    v0a = pnz.tile([P, 512], F32, tag="nz", name="v0a")
                    v0b = pnz.tile([P, 512], F32, tag="nz", name="v0b")
                    v0 = [v0a, v0b]
                num = None
                if ETp is not None:
                    num = pnz.tile([P, 512], F32, tag="nz")
                acc = None
                for g in range(4):
                    for kk in range(2):
                        kc = 2 * g + kk
                        if ET is not None:
                            # scores: key chunk kc, 2-way row tiling over
                            # query halves (K=64 each, M=128 keys, N=512)
                            ps = pss.tile([P, 1024], F32, tag="sc")
                            for pi in range(2):
                                rows = slice(64 * pi, 64 * pi + 64)
                                nc.tensor.matmul(
                                    ps[:, pi * 512:(pi + 1) * 512],
                                    kS3[rows, h, kc * 128:(kc + 1) * 128],
                                    qS2[rows, h, :],
                                    start=True, stop=True,
                                    skip_group_check=True)
                            nc.scalar.activation(
                                ET[:, kc, :, :],
                                ps[:].rearrange("p (a b) -> p a b", a=2),
                                EXP_FN)
                            # running DVE partial sum over chunks (for Z)
                            if kc >= 1:
                                nacc = pes.tile([P, 2, 512], BF16, tag="es")
                                nc.vector.tensor_tensor(
                                    nacc[:], acc[:] if kc > 1 else ET[:, 0],
                                    ET[:, kc], ADD)
                                acc = nacc
                        if v0 is not None:  # head-0 pipeline fill: vproj(0)
                            for f in range(2):
                                nc.tensor.matmul(
                                    v0[f][:], xv[:, kc, 0:P],
                                    wv[:, kc, f * 512:(f + 1) * 512],
                                    start=(kc == 0), stop=(kc == 7),
                                    skip_group_check=True)
                    if ETp is not None:
                        # PV: accumulate over key chunks, col-2x pairs
                        for kk in range(2):
                            kc = 2 * g + kk
                            for pi in range(2):
                                nc.tensor.matmul(
                                    num[64 * pi:64 * pi + 64, :],
                                    vS[:, hp, (kc + 4 * pi) % 8, :],
                                    ETp[:, kc, pi, :],
                                    start=(kc == 0), stop=(kc == 7),
                                    skip_group_check=True)
                    if g == 2 and h < 4:
                        # deferred k-projection (nchunk=1) fills PE idle
                        emit_proj_block(xk, wk, kS3, h, 1)
                        if h == 3:
                            emit_swap(1)
                    if g == 0 and h % 2 == 0 and 2 <= h < NH:
                        emit_vproj(h // 2)
                    if g == 0 and h % 2 == 1 and h >= 5:
                        emit_phaseC((h - 5) // 2)
                    if g == 2 and h == NH:
                        emit_phaseC(6, act_evict=True)
                if ET is not None:
                    es_tiles[h] = acc
                if v0 is not None:
                    evict_vproj_halves([v0[0][:], v0[1][:]], 0)
                if ETp is not None:
                    # Z: one ones-matmul pair on the chunk-summed ET
                    ESp = es_tiles.pop(hp)
                    z = pnz.tile([P, 512], F32, tag="nz")
                    for pi in range(2):
                        nc.tensor.matmul(
                            z[64 * pi:64 * pi + 64, :],
                            ones_bf[:], ESp[:, pi, :],
                            start=True, stop=True, skip_group_check=True)
                    zr = psb.tile([P, 512], F32, tag="zr", bufs=1)
                    nc.vector.reciprocal_approx_fast(zr[:], z[:])
                    nc.vector.tensor_tensor(
                        attn[:, 0:8, 64 * hp:64 * hp + 64],
                        num[:].rearrange("p (a b) -> p a b", a=8),
                        zr[:].rearrange("p (a b) -> p a b", a=8),
                        MULT)
            emit_phaseC(7, act_evict=True)

    nc.compile()
    return nc


def _get_nc():
    if "nc" not in _CACHE:
        _CACHE["nc"] = _build_nc()
    return _CACHE["nc"]


def _prep_inputs(q, k, v, w_q, w_k, w_v, w_o):
    bf = ml_dtypes.bfloat16
    wqT = np.ascontiguousarray((np.asarray(w_q, np.float32) / 8.0).T).astype(bf)
    wkT = np.ascontiguousarray(np.asarray(w_k, np.float32).T).astype(bf)
    wvT = np.ascontiguousarray(np.asarray(w_v, np.float32).T).astype(bf)
    woT = np.ascontiguousarray(np.asarray(w_o, np.float32).T).astype(bf)
    in_maps = []
    for b in range(B):
        in_maps.append({
            "qT": np.ascontiguousarray(np.asarray(q[b], np.float32).T).astype(bf),
            "kT": np.ascontiguousarray(np.asarray(k[b], np.float32).T).astype(bf),
            "vT": np.ascontiguousarray(np.asarray(v[b], np.float32).T).astype(bf),
            "wqT": wqT, "wkT": wkT, "wvT": wvT, "woT": woT,
        })
    return in_maps


def kernel(q, k, v, mask, w_q, w_k, w_v, w_o, **_ignored):
    nc = _get_nc()
    in_maps = _prep_inputs(q, k, v, w_q, w_k, w_v, w_o)
    res = run_bass_kernel_spmd(nc, in_maps, core_ids=list(range(B)))
    return np.stack([res.results[b]["out"] for b in range(B)]).astype(np.float32)
